# revision 10
# baseline (speedup 1.0000x reference)
"""Trainium2 Bass kernel for the windowed-local-attention block.

Contract: kernel(**inputs) takes the FULL unsharded inputs
(x: (8,8192,512) f32, w_q/w_k/w_v: (512,512) f32, b_q/b_k/b_v: (512,) f32)
and returns the full output (8,8192,512) f32.

Strategy: data-parallel over batch. B == n_cores == 8, attention is
strictly within a batch element, so each core independently processes
one (8192, 512) sequence; no collectives.

Algorithm (per core), building on two algebraic folds:
  1. Fused QK: with zero q/k biases, S = x (Wq^T Wk scale) x^T = x M x^T,
     so the k-projection disappears and raw x^T serves as the key tensor.
  2. S^T orientation: compute S^T[k, q] tiles (keys on partitions) via
     lhsT = x^T (keys), rhs = T^T (queries). exp(S^T) is then directly
     the lhsT of the O matmul -- no PE transposes, no P^T copies. The
     softmax denominator comes from tiny F=1 ones-matmuls over exp(S^T).

Precision (validated host-side against the jax reference, absmax
2.63e-2 vs budget 3.22e-2): the T projection (T^T = M^T x^T) and the
S matmuls run in fp8-e4m3 with DoubleRow perf mode (K=256 per pass,
2x MAC rate); M is pre-scaled by 2^11 so its ~4e-4-magnitude entries
survive fp8, and exp() un-scales via its scale operand. The V
projection and O = P V matmuls stay bf16 (fp8 there fails the error
budget). Output is stored fp16 (halves store traffic; negligible err).

Per-window PE streaming: T 512ns + S 256 + V 1024 + O 512 + l ~50
= ~2.35us/window at the ~2.0 GHz sustained PE clock -> ~151us vs
200.7us for the all-bf16 version. S-tiles are key-window-paired
(queries {k, k+1} in one F=256 pass) so every DoubleRow LDWEIGHTS
(213ns) hides under a >=256-cycle stream; the tile for the last key
window of each supertile is deferred into the next supertile where the
following window's T-projection exists.
"""

import os
import sys

import numpy as np

for _p in ("/opt/trn_rl_repo",):
    if _p not in sys.path and os.path.isdir(_p):
        sys.path.insert(0, _p)

import ml_dtypes

import concourse.bass as bass
import concourse.mybir as mybir
import concourse.tile as tile
from concourse import bacc
from concourse.bass import ds
from concourse.bass_utils import run_bass_kernel_spmd

FP32 = mybir.dt.float32
FP16 = mybir.dt.float16
BF16 = mybir.dt.bfloat16
FP8 = mybir.dt.float8e4
DR = mybir.MatmulPerfMode.DoubleRow

D = 512          # model dim
WS = 128         # attention window size
ST = 512         # tokens per supertile (4 windows)
XW = ST + WS     # x tile width incl. 1-window left halo
NCORES = 8
NEG = -1.0e9     # mask fill (pre-exp-scale; exp(scale*NEG) == 0.0)
MSCALE = 2048.0  # 2^11: lifts M (~4e-4) and T (~9e-3) into fp8 range

S_FP8 = True     # S-path in fp8-DoubleRow (False: T-proj fp8, S bf16)


def build_nc(ntok: int):
    """Build + compile the per-core Bass program for `ntok` tokens."""
    nst = ntok // ST
    nwin = ntok // WS
    qm_dt = FP8 if S_FP8 else BF16
    exp_scale = (1.0 / MSCALE) if S_FP8 else 1.0

    nc = bacc.Bacc(
        "TRN2", target_bir_lowering=False, debug=False, num_devices=NCORES
    )

    x8_d = nc.dram_tensor("x8", [D, ntok], FP8, kind="ExternalInput").ap()
    xb_d = nc.dram_tensor("xb", [D, ntok], BF16, kind="ExternalInput").ap()
    mm_d = nc.dram_tensor("mqk", [D, D], FP8, kind="ExternalInput").ap()
    wv_d = nc.dram_tensor("wvT", [D, D], BF16, kind="ExternalInput").ap()
    # [k, q] triangle: NEG where key > query (same-window causal mask)
    mask_d = nc.dram_tensor("mask", [WS, WS], FP32, kind="ExternalInput").ap()
    out_d = nc.dram_tensor("out", [ntok, D], FP16, kind="ExternalOutput").ap()

    with tile.TileContext(nc) as tc:
        with (
            tc.tile_pool(name="const", bufs=1) as cpool,
            tc.tile_pool(name="sb", bufs=2) as sb,
            tc.tile_pool(name="ps", bufs=2, space="PSUM") as ps,
        ):
            # PE warmup: dummy matmuls overlap the head DMAs so the HAM
            # clock-gate reaches full rate before the first real matmul.
            warm = cpool.tile([128, D], BF16, name="warm")
            nc.vector.memset(warm, 0.0)
            wps = ps.tile([128, D], FP32, tag="pv", name="wps")
            for _ in range(8):
                nc.tensor.matmul(
                    wps, lhsT=warm[:, 0:128], rhs=warm, start=True, stop=True
                )
            nc.scalar.copy(warm[:, 0:1], wps[:, 0:1])

            def load_x(t, dram, dt, name, queues):
                xt = sb.tile([128, 4, XW], dt, tag=name, bufs=4, name=f"{name}{t}")
                if t == 0:
                    nc.vector.memset(xt[:, :, ds(0, WS)], 0.0)
                    src = dram[:, ds(0, ST)].rearrange("(c p) s -> p c s", p=128)
                    for c in range(4):
                        queues[c % len(queues)].dma_start(
                            out=xt[:, c, ds(WS, ST)], in_=src[:, c, :]
                        )
                else:
                    src = dram[:, ds(t * ST - WS, XW)].rearrange(
                        "(c p) s -> p c s", p=128
                    )
                    for c in range(4):
                        queues[c % len(queues)].dma_start(
                            out=xt[:, c, :], in_=src[:, c, :]
                        )
                return xt

            # head: first supertile's x + M chunks first so the first
            # projection matmuls start early; wv/xtb follow
            mm_sb = cpool.tile([128, 4, D], FP8, name="mm_sb")
            mm_r = mm_d.rearrange("(c p) d -> p c d", p=128)
            xt8_0 = load_x(0, x8_d, FP8, "x8", (nc.sync, nc.gpsimd, nc.scalar, nc.sync))
            nc.gpsimd.dma_start(out=mm_sb[:, 0, :], in_=mm_r[:, 0, :])
            nc.scalar.dma_start(out=mm_sb[:, 1, :], in_=mm_r[:, 1, :])
            nc.gpsimd.dma_start(out=mm_sb[:, 2, :], in_=mm_r[:, 2, :])
            nc.sync.dma_start(out=mm_sb[:, 3, :], in_=mm_r[:, 3, :])

            wv_sb = cpool.tile([128, 4, D], BF16, name="wv_sb")
            wv_r = wv_d.rearrange("(c p) d -> p c d", p=128)
            for c in range(4):
                (nc.scalar if c % 2 else nc.gpsimd).dma_start(
                    out=wv_sb[:, c, :], in_=wv_r[:, c, :]
                )
            xtb_0 = load_x(0, xb_d, BF16, "xb", (nc.scalar, nc.gpsimd, nc.sync, nc.scalar))
            mask_sb = cpool.tile([128, WS], FP32, name="mask_sb")
            nc.gpsimd.dma_start(out=mask_sb, in_=mask_d.rearrange("k q -> k q"))
            ones_sb = cpool.tile([128, 1], BF16, name="ones_sb")
            nc.vector.memset(ones_sb, 1.0)

            # per-window state (rolling)
            vs: dict[int, object] = {}
            ptks: dict[int, object] = {}

            def emit_qm(t, xt8, xtb):
                """T^T projection for this supertile's 512 query tokens."""
                qm = sb.tile([128, 4, ST], qm_dt, tag="qm", bufs=2, name=f"qm{t}")
                for m in range(4):
                    pq = ps.tile([128, ST], FP32, tag="pq", name=f"pq{t}_{m}")
                    for cp in range(2):
                        nc.tensor.matmul(
                            pq,
                            lhsT=mm_sb[:, slice(2 * cp, 2 * cp + 2), ds(m * 128, 128)],
                            rhs=xt8[:, slice(2 * cp, 2 * cp + 2), ds(WS, ST)],
                            start=(cp == 0),
                            stop=(cp == 1),
                            perf_mode=DR,
                        )
                    nc.scalar.copy(qm[:, m, :], pq)
                return qm

            def emit_v(t, j, xtb):
                """V = x Wv^T for window t*4+j (token-major, bf16)."""
                w = t * 4 + j
                pv = ps.tile([128, D], FP32, tag="pv", name=f"pv{w}")
                for c in range(4):
                    nc.tensor.matmul(
                        pv,
                        lhsT=xtb[:, c, ds((j + 1) * WS, WS)],
                        rhs=wv_sb[:, c, :],
                        start=(c == 0),
                        stop=(c == 3),
                    )
                vj = sb.tile([128, D], BF16, tag="v", bufs=6, name=f"v{w}")
                nc.vector.tensor_scalar_mul(vj, pv, 1.0)
                vs[w] = vj

            ncp = 2 if S_FP8 else 4

            def cslice(cp):
                return slice(2 * cp, 2 * cp + 2) if S_FP8 else cp

            def emit_stk(t, j, xk, qm, qn=2 * WS):
                """S^T tile for key-window k=t*4+j, queries {k .. k+qn/WS-1}.

                stk cols 0:WS   = queries k   (same window -> triangle mask)
                stk cols WS:2WS = queries k+1 (key window is their lookback)
                col 2*WS rides along in the same bank as the l accumulator.
                """
                k = t * 4 + j
                stk = ps.tile([128, 2 * WS + 4], FP32, tag="s", name=f"stk{k}")
                for cp in range(ncp):
                    nc.tensor.matmul(
                        stk[:, ds(0, qn)],
                        lhsT=xk[:, cslice(cp), ds((j + 1) * WS, WS)],
                        rhs=qm[:, cslice(cp), ds(j * WS, qn)],
                        start=(cp == 0),
                        stop=(cp == ncp - 1),
                        perf_mode=DR if S_FP8 else None,
                    )
                finish_stk(k, stk, qn)

            def emit_stk_deferred(t, xk, qm_prev, qm):
                """S^T tile for key-window k=t*4-1 (keys in xt8's halo):
                queries k from qm_prev's last window, queries k+1 from qm."""
                k = t * 4 - 1
                stk = ps.tile([128, 2 * WS + 4], FP32, tag="s", name=f"stk{k}")
                for cp in range(ncp):
                    nc.tensor.matmul(
                        stk[:, ds(0, WS)],
                        lhsT=xk[:, cslice(cp), ds(0, WS)],
                        rhs=qm_prev[:, cslice(cp), ds(3 * WS, WS)],
                        start=(cp == 0),
                        stop=(cp == ncp - 1),
                        perf_mode=DR if S_FP8 else None,
                    )
                for cp in range(ncp):
                    nc.tensor.matmul(
                        stk[:, ds(WS, WS)],
                        lhsT=xk[:, cslice(cp), ds(0, WS)],
                        rhs=qm[:, cslice(cp), ds(0, WS)],
                        start=(cp == 0),
                        stop=(cp == ncp - 1),
                        perf_mode=DR if S_FP8 else None,
                    )
                finish_stk(k, stk, 2 * WS)

            def finish_stk(k, stk, qn):
                # causal triangle on the same-window half, then exp
                nc.vector.tensor_add(
                    out=stk[:, ds(0, WS)], in0=stk[:, ds(0, WS)], in1=mask_sb
                )
                ptk = sb.tile([128, 2 * WS], BF16, tag="ptk", bufs=3, name=f"ptk{k}")
                nc.scalar.activation(
                    out=ptk[:, ds(0, qn)],
                    in_=stk[:, ds(0, qn)],
                    func=mybir.ActivationFunctionType.Exp,
                    bias=0.0,
                    scale=exp_scale,
                )
                ptks[k] = (ptk, stk)

            def emit_o(w):
                """O_w = P_w V: prev-key half from ptk[w-1], cur from ptk[w]."""
                o = ps.tile([128, D], FP32, tag="o", name=f"o{w}")
                lps = ptks[w][1][:, ds(2 * WS, 1)]
                parts = []
                if w > 0:
                    parts.append((ptks[w - 1][0][:, ds(WS, WS)], vs[w - 1]))
                parts.append((ptks[w][0][:, ds(0, WS)], vs[w]))
                n = len(parts)
                for i, (pt, v) in enumerate(parts):
                    nc.tensor.matmul(
                        o, lhsT=pt, rhs=v, start=(i == 0), stop=(i == n - 1)
                    )
                for i, (pt, v) in enumerate(parts):
                    nc.tensor.matmul(
                        lps, lhsT=pt, rhs=ones_sb, start=(i == 0), stop=(i == n - 1)
                    )
                r = sb.tile([128, 1], FP32, tag="r", bufs=6, name=f"r{w}")
                nc.vector.reciprocal(r, lps)
                osb = sb.tile([128, D], FP16, tag="osb", bufs=6, name=f"osb{w}")
                if w % 2 == 0:
                    nc.vector.tensor_scalar_mul(osb, o, r)
                else:
                    nc.scalar.mul(osb, o, r)
                st_eng = nc.gpsimd if w % 2 == 0 else nc.sync
                st_eng.dma_start(out=out_d[ds(w * WS, WS), :], in_=osb)

            qm_prev = None
            for t in range(nst):
                if t == 0:
                    xt8, xtb = xt8_0, xtb_0
                else:
                    xt8 = load_x(t, x8_d, FP8, "x8", (nc.sync, nc.gpsimd, nc.scalar, nc.sync))
                    xtb = load_x(t, xb_d, BF16, "xb", (nc.scalar, nc.sync, nc.gpsimd, nc.scalar))
                w0 = t * 4

                xk = xt8 if S_FP8 else xtb
                qm = emit_qm(t, xt8, xtb)
                if t > 0:
                    emit_stk_deferred(t, xk, qm_prev, qm)  # ptk[w0-1]
                emit_v(t, 0, xtb)
                if t > 0:
                    emit_o(w0 - 1)
                emit_stk(t, 0, xk, qm)                     # ptk[w0]
                emit_v(t, 1, xtb)
                emit_o(w0)
                emit_stk(t, 1, xk, qm)                     # ptk[w0+1]
                emit_v(t, 2, xtb)
                emit_o(w0 + 1)
                emit_stk(t, 2, xk, qm)                     # ptk[w0+2]
                emit_v(t, 3, xtb)
                emit_o(w0 + 2)
                qm_prev = qm

            # epilogue: last key-window tile (queries {nwin-1} only)
            k = nwin - 1
            t = nst - 1
            stk = ps.tile([128, 2 * WS + 4], FP32, tag="s", name=f"stk{k}")
            for cp in range(ncp):
                nc.tensor.matmul(
                    stk[:, ds(0, WS)],
                    lhsT=(xt8 if S_FP8 else xtb)[:, cslice(cp), ds(4 * WS, WS)],
                    rhs=qm_prev[:, cslice(cp), ds(3 * WS, WS)],
                    start=(cp == 0),
                    stop=(cp == ncp - 1),
                    perf_mode=DR if S_FP8 else None,
                )
            finish_stk(k, stk, WS)
            emit_o(k)

    nc.compile()
    return nc


_NC_CACHE: dict[int, object] = {}


def _get_nc(ntok: int):
    if ntok not in _NC_CACHE:
        _NC_CACHE[ntok] = build_nc(ntok)
    return _NC_CACHE[ntok]


def _host_prep(x, w_q, w_k, w_v):
    """Build the per-core input maps (host-side shard + preprocess)."""
    bf = ml_dtypes.bfloat16
    f8 = ml_dtypes.float8_e4m3
    b, ntok, d = x.shape
    assert d == D
    scale = float(d) ** -0.5

    mqk = (w_q.astype(np.float64).T @ w_k.astype(np.float64) * scale).astype(
        np.float64
    )
    mm8 = np.clip(mqk * MSCALE, -240.0, 240.0).astype(np.float32).astype(f8)
    wv = np.ascontiguousarray(w_v.T).astype(bf)

    # [k, q]: NEG where key strictly after query (same-window causal)
    kk = np.arange(WS)
    mask = np.where(kk[:, None] > kk[None, :], NEG, 0.0).astype(np.float32)

    in_maps = []
    for i in range(b):
        xT = np.ascontiguousarray(x[i].T)
        in_maps.append(
            {
                "x8": np.clip(xT, -240.0, 240.0).astype(f8),
                "xb": xT.astype(bf),
                "mqk": mm8,
                "wvT": wv,
                "mask": mask,
            }
        )
    return in_maps


def _np_fallback(x, w_q, b_q, w_k, b_k, w_v, b_v):
    """Exact numpy implementation (only used for nonzero biases)."""
    b, ntok, d = x.shape
    nw = ntok // WS
    out = np.empty_like(x, dtype=np.float32)
    scale = d**-0.5
    for i in range(b):
        q = x[i] @ w_q.T + b_q
        k = x[i] @ w_k.T + b_k
        v = x[i] @ w_v.T + b_v
        for w in range(nw):
            qs = q[w * WS : (w + 1) * WS]
            lo = (w - 1) * WS
            ks = k[max(lo, 0) : (w + 1) * WS]
            vsl = v[max(lo, 0) : (w + 1) * WS]
            sim = qs @ ks.T * scale
            tq = np.arange(w * WS, (w + 1) * WS)[:, None]
            tk = np.arange(max(lo, 0), (w + 1) * WS)[None, :]
            sim = np.where(tq < tk, -np.finfo(np.float32).max, sim)
            sim = sim - sim.max(-1, keepdims=True)
            e = np.exp(sim)
            p = e / e.sum(-1, keepdims=True)
            out[i, w * WS : (w + 1) * WS] = p @ vsl
    return out


def run_on_hw(x, w_q, b_q, w_k, b_k, w_v, b_v, trace=False):
    """Run on the 8 NeuronCores; returns (output, BassKernelResults)."""
    b, ntok, _ = x.shape
    assert b == NCORES, f"batch {b} != {NCORES} cores"
    nc = _get_nc(ntok)
    in_maps = _host_prep(x, w_q, w_k, w_v)
    res = run_bass_kernel_spmd(nc, in_maps, list(range(NCORES)), trace=trace)
    out = np.stack(
        [res.results[i]["out"].astype(np.float32) for i in range(NCORES)]
    )
    return out, res


def kernel(x, w_q, b_q, w_k, b_k, w_v, b_v):
    x = np.asarray(x, np.float32)
    w_q = np.asarray(w_q, np.float32)
    b_q = np.asarray(b_q, np.float32)
    w_k = np.asarray(w_k, np.float32)
    b_k = np.asarray(b_k, np.float32)
    w_v = np.asarray(w_v, np.float32)
    b_v = np.asarray(b_v, np.float32)
    if np.any(b_q != 0) or np.any(b_k != 0) or np.any(b_v != 0):
        # the fused-QK kernel folds Wq^T Wk and drops all bias adds;
        # nonzero biases (never the case for this problem's reference
        # inputs) take the exact numpy path
        return _np_fallback(x, w_q, b_q, w_k, b_k, w_v, b_v)
    out, _ = run_on_hw(x, w_q, b_q, w_k, b_k, w_v, b_v)
    return out
